# revision 2
# baseline (speedup 1.0000x reference)
"""Conditional VQ embedding forward on 8 trn2 NeuronCores.

Data-parallel over batch: 4 batches per core. Per batch b, per n-tile of 128
positions:
  s[n,k]  = z[b,n,:] . cb[b,k,:] via 3-pass bf16 hi/lo split matmuls
            (zh.eh + zh.el + zl.eh, fp32 PSUM accumulate). Reproduces the
            fp32 score to ~1e-8 - far below the reference's fp32 rounding
            grid, so argmin picks match the reference (verified: 1 flip
            of 131072 positions).
  v[n,k]  = fp32(2*s - ||z_n||^2)  (ACT Identity, per-partition bias) -
            replicates the reference's fp32 rounding of the distance, whose
            ~2^-15 quantization grid creates index ties that are load-bearing
            (~2% of picks).
  idx[n]  = argmax_k v, first index on ties (DVE max8 + max_index).
The device emits ONLY the u32 winner index per position; the host gathers
the fp32 codewords and replicates the reference's straight-through output
arithmetic exactly (z + (quant - z) in fp32), so the only error source is
index flips (~1 of 131072). This removes the SWDGE indirect gather
(141us of gpsimd time), 17MB of DMA, and the output drain tail that
dominated the old kernel's overhead beyond the ~170us matmul stream.
"""

import numpy as np

B, D, HW, K = 32, 256, 4096, 512
NCORES, BPC = 8, 4
P = 128
NT = HW // P  # 32 n-tiles of 128 per batch

GW = 512  # n-columns per z-DMA group (4 tiles): fine enough to start the
NG = HW // GW  # first matmul after ~256KB of input instead of ~1.7MB
TPG = GW // P

TRACE = False
LAST_RESULT = None
_NC_CACHE = {}


def _build():
    from contextlib import ExitStack

    import concourse.mybir as mybir
    from concourse import bacc
    from concourse.tile import TileContext

    f32 = mybir.dt.float32
    bf16 = mybir.dt.bfloat16
    u32 = mybir.dt.uint32

    nc = bacc.Bacc("TRN2", target_bir_lowering=False, debug=False, num_devices=NCORES)
    zh_in = nc.dram_tensor("zh", [BPC, D, HW], bf16, kind="ExternalInput")
    zl_in = nc.dram_tensor("zl", [BPC, D, HW], bf16, kind="ExternalInput")
    ch_in = nc.dram_tensor("chT", [BPC, D, K], bf16, kind="ExternalInput")
    cl_in = nc.dram_tensor("clT", [BPC, D, K], bf16, kind="ExternalInput")
    an_in = nc.dram_tensor("an", [BPC, HW], f32, kind="ExternalInput")
    idx_out = nc.dram_tensor("idx", [BPC, HW], u32, kind="ExternalOutput")

    with TileContext(nc) as tc, ExitStack() as ctx:
        cb_p = ctx.enter_context(tc.tile_pool(name="cbp", bufs=2))
        an_p = ctx.enter_context(tc.tile_pool(name="anp", bufs=2))
        z_p = ctx.enter_context(tc.tile_pool(name="zp", bufs=3))
        v_p = ctx.enter_context(tc.tile_pool(name="vp", bufs=4))
        m_p = ctx.enter_context(tc.tile_pool(name="mp", bufs=8))
        i_p = ctx.enter_context(tc.tile_pool(name="ip", bufs=3))
        ps_p = ctx.enter_context(tc.tile_pool(name="psp", bufs=6, space="PSUM"))

        for b in range(BPC):
            # issue codebook pieces in first-use order so the first matmul
            # only waits on ch0 + the first z d-chunk
            ch0 = cb_p.tile([P, K], bf16, tag="ch0")
            nc.sync.dma_start(ch0[:], ch_in[b, 0:P, :])
            cl0 = cb_p.tile([P, K], bf16, tag="cl0")
            nc.sync.dma_start(cl0[:], cl_in[b, 0:P, :])
            ch1 = cb_p.tile([P, K], bf16, tag="ch1")
            nc.sync.dma_start(ch1[:], ch_in[b, P : 2 * P, :])
            cl1 = cb_p.tile([P, K], bf16, tag="cl1")
            nc.sync.dma_start(cl1[:], cl_in[b, P : 2 * P, :])
            an_all = an_p.tile([P, NT], f32, tag="an")
            nc.sync.dma_start(an_all[:], an_in[b, :].rearrange("(t p) -> p t", p=P))

            for g in range(NG):
                gs = slice(g * GW, (g + 1) * GW)
                # split z DMAs by d-chunk so the first matmul of the group
                # waits on a 128KB transfer, not 512KB
                zh0 = z_p.tile([P, GW], bf16, tag="zh0")
                nc.sync.dma_start(zh0[:], zh_in[b, 0:P, gs])
                zh1 = z_p.tile([P, GW], bf16, tag="zh1")
                nc.sync.dma_start(zh1[:], zh_in[b, P : 2 * P, gs])
                zl0 = z_p.tile([P, GW], bf16, tag="zl0")
                nc.sync.dma_start(zl0[:], zl_in[b, 0:P, gs])
                zl1 = z_p.tile([P, GW], bf16, tag="zl1")
                nc.sync.dma_start(zl1[:], zl_in[b, P : 2 * P, gs])
                ig = i_p.tile([P, TPG, 8], u32, tag="ig")
                for u in range(TPG):
                    t = g * TPG + u
                    us = slice(u * P, (u + 1) * P)

                    ps = ps_p.tile([P, K], f32, space="PSUM", tag="ps")
                    nc.tensor.matmul(ps[:], lhsT=zh0[:, us], rhs=ch0[:], start=True, stop=False)
                    nc.tensor.matmul(ps[:], lhsT=zh0[:, us], rhs=cl0[:], start=False, stop=False)
                    nc.tensor.matmul(ps[:], lhsT=zl0[:, us], rhs=ch0[:], start=False, stop=False)
                    nc.tensor.matmul(ps[:], lhsT=zh1[:, us], rhs=ch1[:], start=False, stop=False)
                    nc.tensor.matmul(ps[:], lhsT=zh1[:, us], rhs=cl1[:], start=False, stop=False)
                    nc.tensor.matmul(ps[:], lhsT=zl1[:, us], rhs=ch1[:], start=False, stop=True)

                    v = v_p.tile([P, K], f32, tag="v")
                    nc.scalar.activation(
                        out=v[:], in_=ps[:],
                        func=mybir.ActivationFunctionType.Identity,
                        bias=an_all[:, t : t + 1], scale=2.0,
                    )
                    m8 = m_p.tile([P, 8], f32, tag="m8")
                    nc.vector.max(out=m8[:], in_=v[:])
                    nc.vector.max_index(out=ig[:, u, :], in_max=m8[:], in_values=v[:])

                nc.sync.dma_start(
                    out=idx_out[b, gs].rearrange("(t p) -> p t", p=P),
                    in_=ig[:, :, 0],
                )

    nc.compile()
    return nc


def _get_nc():
    if "nc" not in _NC_CACHE:
        _NC_CACHE["nc"] = _build()
    return _NC_CACHE["nc"]


def kernel(z_e_x, C, weight):
    global LAST_RESULT
    import ml_dtypes
    from concourse.bass_utils import run_bass_kernel_spmd

    z_e_x = np.asarray(z_e_x, dtype=np.float32)
    C = np.asarray(C).astype(np.int64)
    weight = np.asarray(weight, dtype=np.float32)

    # ||z_n||^2 computed with the exact op sequence of the reference on the
    # default jax backend, so the fp32 bits match the reference's dist term.
    import jax.numpy as jnp

    zj = jnp.asarray(z_e_x)
    zr_j = jnp.transpose(zj, (0, 2, 3, 1)).reshape(B, HW, D)
    A = jnp.sum(zr_j * zr_j, axis=-1, keepdims=True)
    an = -np.asarray(A)[..., 0]  # [B, HW] fp32, negated for the ACT bias

    zflat = z_e_x.reshape(B, D, HW)
    zh = zflat.astype(ml_dtypes.bfloat16)
    zl = (zflat - zh.astype(np.float32)).astype(ml_dtypes.bfloat16)

    cb_all = weight[C]  # [B, K, D] fp32
    ch = cb_all.astype(ml_dtypes.bfloat16)
    cl = (cb_all - ch.astype(np.float32)).astype(ml_dtypes.bfloat16)
    chT = np.ascontiguousarray(np.swapaxes(ch, 1, 2))  # [B, D, K] bf16
    clT = np.ascontiguousarray(np.swapaxes(cl, 1, 2))

    nc = _get_nc()
    in_maps = []
    for c in range(NCORES):
        bs = slice(c * BPC, (c + 1) * BPC)
        in_maps.append(
            dict(
                zh=np.ascontiguousarray(zh[bs]),
                zl=np.ascontiguousarray(zl[bs]),
                chT=chT[bs],
                clT=clT[bs],
                an=np.ascontiguousarray(an[bs]).astype(np.float32),
            )
        )
    res = run_bass_kernel_spmd(nc, in_maps, core_ids=list(range(NCORES)), trace=TRACE)
    LAST_RESULT = res
    idx = np.concatenate([np.asarray(r["idx"]) for r in res.results], 0)  # [B, HW] u32

    # Host-side compose: gather fp32 codewords and replicate the reference's
    # straight-through arithmetic bit-for-bit (IEEE fp32 elementwise).
    zr = np.ascontiguousarray(z_e_x.transpose(0, 2, 3, 1)).reshape(B, HW, D)
    quant = np.take_along_axis(cb_all, idx.astype(np.int64)[:, :, None], axis=1)
    z_q = zr + (quant - zr)
    z_q_x = np.ascontiguousarray(z_q.reshape(B, 64, 64, D).transpose(0, 3, 1, 2))
    z_q_x_bar = np.ascontiguousarray(quant.reshape(B, 64, 64, D).transpose(0, 3, 1, 2))
    return z_q_x, z_q_x_bar


# revision 5
# speedup vs baseline: 1.2539x; 1.2539x over previous
"""Conditional VQ embedding forward on 8 trn2 NeuronCores.

Data-parallel over batch: 4 batches per core. Per batch b, per n-tile of 128
positions:
  s[n,k]  = z[b,n,:] . cb[b,k,:] via 3-pass bf16 hi/lo split matmuls
            (zh.eh + zh.el + zl.eh, fp32 PSUM accumulate). Reproduces the
            fp32 score to ~1e-8 - far below the reference's fp32 rounding
            grid, so argmin picks match the reference (verified: 1 flip
            of 131072 positions).
  v[n,k]  = fp32(2*s - ||z_n||^2)  (ACT Identity, per-partition bias) -
            replicates the reference's fp32 rounding of the distance, whose
            ~2^-15 quantization grid creates index ties that are load-bearing
            (~2% of picks).
  idx[n]  = argmax_k v, first index on ties (DVE max8 + max_index).
The device emits ONLY the u32 winner index per position; the host gathers
the fp32 codewords and replicates the reference's straight-through output
arithmetic exactly (z + (quant - z) in fp32), so the only error source is
index flips (~1 of 131072). This removes the SWDGE indirect gather
(141us of gpsimd time), 17MB of DMA, and the output drain tail that
dominated the old kernel's overhead beyond the ~170us matmul stream.
"""

import numpy as np

B, D, HW, K = 32, 256, 4096, 512
NCORES, BPC = 8, 4
P = 128
NT = HW // P  # 32 n-tiles of 128 per batch

GW = 2048  # n-columns per z-DMA group; dma_start has ~800ns serial issue
NG = HW // GW  # cost on the sync sequencer, so favor few, large transfers
TPG = GW // P

TRACE = False
LAST_RESULT = None
_NC_CACHE = {}


def _build():
    from contextlib import ExitStack

    import concourse.mybir as mybir
    from concourse import bacc
    from concourse.tile import TileContext

    f32 = mybir.dt.float32
    bf16 = mybir.dt.bfloat16
    u32 = mybir.dt.uint32

    nc = bacc.Bacc("TRN2", target_bir_lowering=False, debug=False, num_devices=NCORES)
    zh_in = nc.dram_tensor("zh", [BPC, D, HW], bf16, kind="ExternalInput")
    zl_in = nc.dram_tensor("zl", [BPC, D, HW], bf16, kind="ExternalInput")
    ch_in = nc.dram_tensor("chT", [BPC, D, K], bf16, kind="ExternalInput")
    cl_in = nc.dram_tensor("clT", [BPC, D, K], bf16, kind="ExternalInput")
    an_in = nc.dram_tensor("an", [BPC, HW], f32, kind="ExternalInput")
    idx_out = nc.dram_tensor("idx", [BPC, HW], u32, kind="ExternalOutput")

    with TileContext(nc) as tc, ExitStack() as ctx:
        cb_p = ctx.enter_context(tc.tile_pool(name="cbp", bufs=2))
        an_p = ctx.enter_context(tc.tile_pool(name="anp", bufs=2))
        z_p = ctx.enter_context(tc.tile_pool(name="zp", bufs=3))
        v_p = ctx.enter_context(tc.tile_pool(name="vp", bufs=4))
        m_p = ctx.enter_context(tc.tile_pool(name="mp", bufs=8))
        i_p = ctx.enter_context(tc.tile_pool(name="ip", bufs=2))
        ps_p = ctx.enter_context(tc.tile_pool(name="psp", bufs=6, space="PSUM"))

        for b in range(BPC):
            # issue codebook pieces in first-use order so the first matmul
            # only waits on ch0 + the first z d-chunk
            ch0 = cb_p.tile([P, K], bf16, tag="ch0")
            nc.sync.dma_start(ch0[:], ch_in[b, 0:P, :])
            cl0 = cb_p.tile([P, K], bf16, tag="cl0")
            nc.sync.dma_start(cl0[:], cl_in[b, 0:P, :])
            ch1 = cb_p.tile([P, K], bf16, tag="ch1")
            nc.sync.dma_start(ch1[:], ch_in[b, P : 2 * P, :])
            cl1 = cb_p.tile([P, K], bf16, tag="cl1")
            nc.sync.dma_start(cl1[:], cl_in[b, P : 2 * P, :])
            an_all = an_p.tile([P, NT], f32, tag="an")
            nc.sync.dma_start(an_all[:], an_in[b, :].rearrange("(t p) -> p t", p=P))

            ig = i_p.tile([P, NT, 8], u32, tag="ig")
            for g in range(NG):
                gs = slice(g * GW, (g + 1) * GW)
                zh_g = z_p.tile([P, 2, GW], bf16, tag="zh")
                nc.sync.dma_start(zh_g[:], zh_in[b, :, gs].rearrange("(c p) n -> p c n", p=P))
                zl_g = z_p.tile([P, 2, GW], bf16, tag="zl")
                nc.sync.dma_start(zl_g[:], zl_in[b, :, gs].rearrange("(c p) n -> p c n", p=P))
                for u in range(TPG):
                    t = g * TPG + u
                    us = slice(u * P, (u + 1) * P)

                    ps = ps_p.tile([P, K], f32, space="PSUM", tag="ps")
                    nc.tensor.matmul(ps[:], lhsT=zh_g[:, 0, us], rhs=ch0[:], start=True, stop=False)
                    nc.tensor.matmul(ps[:], lhsT=zh_g[:, 0, us], rhs=cl0[:], start=False, stop=False)
                    nc.tensor.matmul(ps[:], lhsT=zl_g[:, 0, us], rhs=ch0[:], start=False, stop=False)
                    nc.tensor.matmul(ps[:], lhsT=zh_g[:, 1, us], rhs=ch1[:], start=False, stop=False)
                    nc.tensor.matmul(ps[:], lhsT=zh_g[:, 1, us], rhs=cl1[:], start=False, stop=False)
                    nc.tensor.matmul(ps[:], lhsT=zl_g[:, 1, us], rhs=ch1[:], start=False, stop=True)

                    v = v_p.tile([P, K], f32, tag="v")
                    nc.scalar.activation(
                        out=v[:], in_=ps[:],
                        func=mybir.ActivationFunctionType.Identity,
                        bias=an_all[:, t : t + 1], scale=2.0,
                    )
                    m8 = m_p.tile([P, 8], f32, tag="m8")
                    nc.vector.max(out=m8[:], in_=v[:])
                    nc.vector.max_index(out=ig[:, t, :], in_max=m8[:], in_values=v[:])

            nc.sync.dma_start(
                out=idx_out[b, :].rearrange("(t p) -> p t", p=P),
                in_=ig[:, :, 0],
            )

    nc.compile()
    return nc


def _get_nc():
    if "nc" not in _NC_CACHE:
        _NC_CACHE["nc"] = _build()
    return _NC_CACHE["nc"]


def kernel(z_e_x, C, weight):
    global LAST_RESULT
    import ml_dtypes
    from concourse.bass_utils import run_bass_kernel_spmd

    z_e_x = np.asarray(z_e_x, dtype=np.float32)
    C = np.asarray(C).astype(np.int64)
    weight = np.asarray(weight, dtype=np.float32)

    # ||z_n||^2 computed with the exact op sequence of the reference on the
    # default jax backend, so the fp32 bits match the reference's dist term.
    import jax.numpy as jnp

    zj = jnp.asarray(z_e_x)
    zr_j = jnp.transpose(zj, (0, 2, 3, 1)).reshape(B, HW, D)
    A = jnp.sum(zr_j * zr_j, axis=-1, keepdims=True)
    an = -np.asarray(A)[..., 0]  # [B, HW] fp32, negated for the ACT bias

    zflat = z_e_x.reshape(B, D, HW)
    zh = zflat.astype(ml_dtypes.bfloat16)
    zl = (zflat - zh.astype(np.float32)).astype(ml_dtypes.bfloat16)

    cb_all = weight[C]  # [B, K, D] fp32
    ch = cb_all.astype(ml_dtypes.bfloat16)
    cl = (cb_all - ch.astype(np.float32)).astype(ml_dtypes.bfloat16)
    chT = np.ascontiguousarray(np.swapaxes(ch, 1, 2))  # [B, D, K] bf16
    clT = np.ascontiguousarray(np.swapaxes(cl, 1, 2))

    nc = _get_nc()
    in_maps = []
    for c in range(NCORES):
        bs = slice(c * BPC, (c + 1) * BPC)
        in_maps.append(
            dict(
                zh=np.ascontiguousarray(zh[bs]),
                zl=np.ascontiguousarray(zl[bs]),
                chT=chT[bs],
                clT=clT[bs],
                an=np.ascontiguousarray(an[bs]).astype(np.float32),
            )
        )
    res = run_bass_kernel_spmd(nc, in_maps, core_ids=list(range(NCORES)), trace=TRACE)
    LAST_RESULT = res
    idx = np.concatenate([np.asarray(r["idx"]) for r in res.results], 0)  # [B, HW] u32

    # Host-side compose: gather fp32 codewords and replicate the reference's
    # straight-through arithmetic bit-for-bit (IEEE fp32 elementwise).
    zr = np.ascontiguousarray(z_e_x.transpose(0, 2, 3, 1)).reshape(B, HW, D)
    quant = np.take_along_axis(cb_all, idx.astype(np.int64)[:, :, None], axis=1)
    z_q = zr + (quant - zr)
    z_q_x = np.ascontiguousarray(z_q.reshape(B, 64, 64, D).transpose(0, 3, 1, 2))
    z_q_x_bar = np.ascontiguousarray(quant.reshape(B, 64, 64, D).transpose(0, 3, 1, 2))
    return z_q_x, z_q_x_bar
